# revision 57
# baseline (speedup 1.0000x reference)
"""Trainium2 Bass kernel for nn_CNNFusing (segment_reduce) — v3.

Math (per token t in session b, H=128, L=64 tokens/session):
  mean_b   = (1/L) sum_{t in b} hidden_t
  pos_h_t  = tanh(hidden_t @ Wp1.T + pos_table[rp_t] @ Wp2.T + W_pos_b)
  gate_t   = sigmoid(mean_b @ W1.T + W1_b + pos_h_t @ W2.T + W2_b)
  alpha_t  = gate_t @ q_w.T + q_b
  out_b    = sum_{t in b} alpha_t * hidden_t

Design (measured ~173 us on 8 axon trn2 cores, rel err ~7.7e-3; the
v2 baseline was ~225 us):
  - hidT is transposed on the HOST (no hardware DMA-transpose) with a
    per-block column permutation col = i*128 + s*16 + q' <-> token
    t = s*64 + 4*q' + i, so that:
      * natural-layout loads are fully contiguous (partition p holds 4
        consecutive tokens 4p..4p+3 => 1 KiB runs, full DMA rate)
      * alpha chunk i is a contiguous 128-col stationary slice of gate
        whose output partition order matches the natural-layout partitions
  - The position-table add is folded into the A matmul on the host:
    hidt' = h + Wp1^-1 @ PC[pos].
  - Session means, and hence z1 = W1 @ mean, are computed on the host
    (fp32, improves accuracy) and DMA'd in packed for a 4-WAY ROW-TILED
    z1 broadcast: each block's z1 only needs its 8 sessions as
    contraction, so the 4 blocks of a quad use the four 32-row PE
    groups and their onehot matmuls run concurrently.
  - hid and hidT stream on SEPARATE DMA queues (sync / gpsimd): one
    queue saturates at ~260 GB/s, the pair needs ~360.
  - alpha matmuls write (i*4+b)-ordered columns so ONE fused DVE
    scalar_tensor_tensor per quad builds the masked wsum stationary
    (3-dim APs only — the STT verifier rejects 4-dim).
  - outputs are staged in SBUF, stored once per superblock on the sync
    queue (NOT behind the hidT loads on gpsimd).
  - The engine schedule is ACT-bound (tanh+sigmoid = 2 passes over
    [T, H] at 1 elem/lane/cycle ~= 110 us + per-call overhead): the
    main loop interleaves B(k) parts BEFORE A(k+1) parts so sigmoids
    (which release PSUM tiles and feed the alphas) never queue behind
    a burst of tanhs on the FIFO ACT engine.
  - PE pre-warm via a memset-backed stationary (no DMA dependency)
    covers the HAM clock-gate window during the DMA-queue ramp.
"""

import sys

sys.path.insert(0, "/opt/trn_rl_repo")

import numpy as np
import ml_dtypes

BF16 = ml_dtypes.bfloat16
FP8 = ml_dtypes.float8_e4m3

H = 128
L = 64
NT = 512            # tokens per block
SPB = 8             # sessions per block
SUPER = 8           # blocks per superblock
N_CORES = 8
FP8_A = False  # fp8 DoubleRow for the Wp1 (A) matmul: measured 5.5e-2 rel err
               # (fails 2e-2 gate) and slower than bf16+FWL — keep off

_CACHE = {}


def _build(n_blocks):
    import concourse.bacc as bacc
    import concourse.bass as bass
    import concourse.tile as tile
    from concourse import mybir

    f32 = mybir.dt.float32
    bf16 = mybir.dt.bfloat16
    fp8 = mybir.dt.float8e4
    DR = mybir.MatmulPerfMode.DoubleRow
    Tanh = mybir.ActivationFunctionType.Tanh
    Sig = mybir.ActivationFunctionType.Sigmoid
    X = mybir.AxisListType.X
    XY = mybir.AxisListType.XY
    ADD = mybir.AluOpType.add
    MUL = mybir.AluOpType.mult

    T_core = n_blocks * NT
    assert n_blocks % SUPER == 0
    n_super = n_blocks // SUPER

    nc = bacc.Bacc("TRN2", target_bir_lowering=False, debug=False)

    hid = nc.dram_tensor("hid", [T_core, H], bf16, kind="ExternalInput").ap()
    if FP8_A:
        hidt8 = nc.dram_tensor("hidt8", [64, 2, T_core], fp8, kind="ExternalInput").ap()
        a_drt = nc.dram_tensor("a_dr", [64, 2, H], fp8, kind="ExternalInput").ap()
    else:
        hidt = nc.dram_tensor("hidt", [H, T_core], bf16, kind="ExternalInput").ap()
    z1t4 = nc.dram_tensor(
        "z1t4", [H, n_super * 2 * H], bf16, kind="ExternalInput"
    ).ap()
    cpk = nc.dram_tensor("cpk", [H, 897], bf16, kind="ExternalInput").ap()
    oh4 = nc.dram_tensor("oh4", [H, NT], bf16, kind="ExternalInput").ap()
    bbqb = nc.dram_tensor("bbqb", [H, 2], f32, kind="ExternalInput").ap()
    out = nc.dram_tensor("out", [n_blocks * SPB, H], f32, kind="ExternalOutput").ap()

    with tile.TileContext(nc) as tc:
        with (
            tc.tile_pool(name="consts", bufs=1) as consts,
            tc.tile_pool(name="hidn", bufs=7) as hidn_pool,
            tc.tile_pool(name="hidtp", bufs=5) as hidt_pool,
            tc.tile_pool(name="posh", bufs=12) as posh_pool,
            tc.tile_pool(name="gate", bufs=6) as gate_pool,
            tc.tile_pool(name="smallsb", bufs=3) as smallsb,
            tc.tile_pool(name="acbp", bufs=3) as acb_pool,
            tc.tile_pool(name="osbp", bufs=3) as osb_pool,
            tc.tile_pool(name="dense_ps", bufs=3, space=bass.MemorySpace.PSUM) as dense_ps,
            tc.tile_pool(name="out_ps", bufs=1, space=bass.MemorySpace.PSUM) as out_ps,
            tc.tile_pool(name="small_ps", bufs=1, space=bass.MemorySpace.PSUM) as small_ps,
        ):
    # ---- constants ----
            # PE pre-warm first: a memset-backed stationary means the warm
            # burst has NO DMA dependency and starts in the preamble shadow.
            warm_sb = consts.tile([1, 1], f32)
            wz_sb = consts.tile([H, H], bf16)
            nc.vector.memset(wz_sb, 0.0)
            warm_ps = dense_ps.tile([128, 2, NT], f32, tag="dense")
            for i in range(60):
                nc.tensor.matmul(
                    warm_ps[:, i % 2, 0:128], wz_sb, wz_sb,
                    start=True, stop=True,
                )
            nc.vector.tensor_copy(warm_sb, warm_ps[0:1, 0, 0:1])

            # one packed bf16 const DMA (a | w2 | w1 | mf16 | q), plus three
            # small ones: few issue slots, single descriptor streams
            cpk_sb = consts.tile([H, 897], bf16)
            nc.scalar.dma_start(cpk_sb, cpk)
            a_sb = cpk_sb[:, 0:128]
            w2_sb = cpk_sb[:, 128:256]
            w1_sb = cpk_sb[:, 256:384]
            mf16_sb = cpk_sb[:, 384:896]
            q_sb = cpk_sb[:, 896:897]
            oh4_sb = consts.tile([H, NT], bf16)
            nc.scalar.dma_start(oh4_sb, oh4)
            z1t4_sb = consts.tile([H, n_super, 2, H], bf16)
            nc.scalar.dma_start(
                z1t4_sb.rearrange("p k r m -> p (k r m)"), z1t4
            )
            bbqb_sb = consts.tile([H, 2], f32)
            nc.scalar.dma_start(bbqb_sb, bbqb)
            bb_sb = bbqb_sb[:, 0:1]
            qb_sb = bbqb_sb[:, 1:2]
            if FP8_A:
                adr_sb = consts.tile([64, 2, H], fp8)
                nc.scalar.dma_start(
                    adr_sb.rearrange("p k h -> p (k h)"),
                    a_drt.rearrange("p k h -> p (k h)"),
                )

            state = {}

            def phase_a_parts(k):
                st = {"hidnats": [], "poshs": [], "ht": None}
                state[k] = st

                def mk(qd, pr):
                    def part():
                        if pr == 0:
                            t0 = (k * SUPER + qd * 4) * NT
                            hidnat = hidn_pool.tile([128, 4, 4, H], bf16, tag="hn")
                            nc.sync.dma_start(
                                hidnat.rearrange("p b i h -> p b (i h)"),
                                hid[t0:t0 + 4 * NT, :].rearrange(
                                    "(b p i) h -> p b (i h)", p=128, i=4
                                ),
                            )
                            st["hidnats"].append(hidnat)
                            # separate hardware DMA queue from hidnat's (sync):
                            # one queue saturates at ~260 GB/s
                            if FP8_A:
                                ht = hidt_pool.tile([64, 2, 4 * NT], fp8, tag="ht")
                                nc.gpsimd.dma_start(
                                    ht, hidt8[:, :, t0:t0 + 4 * NT]
                                )
                            else:
                                ht = hidt_pool.tile([H, 4 * NT], bf16, tag="ht")
                                if k == 0 and qd == 0:
                                    # split the very first load so the A
                                    # matmul of block 0 starts ~3us earlier
                                    for blk in range(4):
                                        nc.gpsimd.dma_start(
                                            ht[:, blk * NT:(blk + 1) * NT],
                                            hidt[:, t0 + blk * NT:t0 + (blk + 1) * NT],
                                        )
                                else:
                                    nc.gpsimd.dma_start(
                                        ht, hidt[:, t0:t0 + 4 * NT]
                                    )
                            st["ht"] = ht
                        hidt_sb = st["ht"]
                        zpair = dense_ps.tile([128, 2, NT], f32, tag="dense")
                        for b2 in range(2):
                            off = (pr * 2 + b2) * NT
                            if FP8_A:
                                nc.tensor.matmul(
                                    zpair[:, b2, :], adr_sb,
                                    hidt_sb[:, :, off:off + NT],
                                    start=True, stop=True, perf_mode=DR,
                                )
                            else:
                                nc.tensor.matmul(
                                    zpair[:, b2, :], a_sb,
                                    hidt_sb[:, off:off + NT],
                                    start=True, stop=True,
                                )
                        posh = posh_pool.tile([128, 2, NT], bf16)
                        nc.scalar.activation(posh, zpair, Tanh)
                        st["poshs"].append(posh)
                    return part

                return [mk(qd, pr) for qd in range(2) for pr in range(2)]

            def phase_b_parts(k):
                st = state[k]
                st["pend"] = []
                st["gates"] = {}
                st["zp"] = {}

                def alphas(qd, pr):
                    gate, ab = st["gates"][(qd, pr)]
                    for b2 in range(2):
                        b = pr * 2 + b2  # block in quad
                        for i in range(4):
                            # ab column order (i, b) so the quad_tail STT can
                            # view ab as one uniform-stride (i b) dim
                            nc.tensor.matmul(
                                ab[:, i * 4 + b:i * 4 + b + 1],
                                gate[:, b2, i * 128:(i + 1) * 128], q_sb,
                                start=True, stop=True,
                            )

                def quad_tail(qd):
                    ab = st["abps"][qd]
                    acb = acb_pool.tile([128, 4, H], bf16, tag="acb")
                    # one fused STT for the whole quad: (alpha + qb) * mask.
                    # ab cols are (i b)-ordered and acb's (i, b, s) layout is
                    # contiguous, so both sides collapse to 3-dim APs (the
                    # STT verifier rejects 4-dim ones); mf16_sb pre-tiles the
                    # mask over i.
                    nc.vector.scalar_tensor_tensor(
                        acb.rearrange("p i (b s) -> p (i b) s", b=4),
                        ab[:, :, None].broadcast_to((128, 16, 32)),
                        qb_sb[:, 0:1],
                        mf16_sb.rearrange("p (ib s) -> p ib s", s=32),
                        op0=ADD, op1=MUL,
                    )
                    out4 = out_ps.tile([128, 4, H], f32)
                    for i in range(4):
                        nc.tensor.matmul(
                            out4, acb[:, i, :], st["hidnats"][qd][:, :, i, :],
                            start=(i == 0), stop=(i == 3),
                        )
                    for b in range(4):
                        nc.vector.tensor_copy(
                            st["osb"][32 * b:32 * b + SPB, qd, :],
                            out4[32 * b:32 * b + SPB, b, :],
                        )

                def flush_one():
                    pq = st["pend"].pop(0)
                    alphas(*pq)
                    if pq[1] == 1:
                        quad_tail(pq[0])

                def mk(qd, pr):
                    def part():
                        if qd == 0 and pr == 0:
                            st["osb"] = osb_pool.tile([128, 2, H], f32, name="osb")
                            ab2 = small_ps.tile([128, 32], f32, tag="ab")
                            st["abps"] = [ab2[:, 0:16], ab2[:, 16:32]]
                        posh = st["poshs"][qd * 2 + pr]
                        zpair2 = dense_ps.tile([128, 2, NT], f32, tag="dense")
                        for b2 in range(2):
                            nc.tensor.matmul(
                                zpair2[:, b2, :], w2_sb, posh[:, b2, :],
                                start=True, stop=False,
                            )
                        st["zp"][pr] = zpair2
                        if pr == 1:
                            # 4-way row-tiled z1 broadcast: each block's z1
                            # needs only its 8 sessions as contraction, so
                            # the four blocks of the quad go to the four
                            # 32-row PE groups and run concurrently
                            for g in range(4):
                                prg, b2g = g // 2, g % 2
                                nc.tensor.matmul(
                                    st["zp"][prg][:, b2g, :],
                                    z1t4_sb[32 * g:32 * g + 32, k, qd, :],
                                    oh4_sb[32 * g:32 * g + 32, :],
                                    start=False, stop=True,
                                    tile_position=(32 * g, 0),
                                )
                            # eager per-sigmoid flush on the last super (no
                            # A-phase work left to hide the alpha/wsum
                            # latency behind)
                            lag = 0 if k == n_super - 1 else 1
                            for prg in range(2):
                                gate = gate_pool.tile([128, 2, NT], bf16)
                                nc.scalar.activation(
                                    gate, st["zp"][prg], Sig, bias=bb_sb
                                )
                                st["gates"][(qd, prg)] = (gate, st["abps"][qd])
                                st["pend"].append((qd, prg))
                                if k == n_super - 1:
                                    while len(st["pend"]) > lag:
                                        flush_one()
                            while len(st["pend"]) > lag:
                                flush_one()
                    return part

                def tail():
                    while st["pend"]:
                        flush_one()
                    # sync queue: the gpsimd queue still has hidt loads of
                    # later superblocks in flight; don't queue stores behind
                    # them (the final drain was eating ~10us)
                    ov = out[k * 64:(k + 1) * 64, :].rearrange(
                        "(h g2 s) x -> g2 s h x", h=2, g2=4
                    )
                    for g in range(4):
                        nc.sync.dma_start(
                            ov[g], st["osb"][32 * g:32 * g + SPB, :, :]
                        )
                    del state[k]

                return [mk(qd, pr) for qd in range(2) for pr in range(2)] + [tail]

            # software pipeline with pair-level interleave: A(k+1) pairs
            # alternate with B(k) pairs so every engine sees a steady mix
            # interleave B(k) parts with A(k+1) parts, B first: sigmoids
            # (critical path: they release PSUM tiles and feed the alphas)
            # must not queue behind a burst of tanhs on the FIFO ACT engine
            for p in phase_a_parts(0):
                p()
            for k in range(n_super):
                bp = phase_b_parts(k)
                ap = phase_a_parts(k + 1) if k + 1 < n_super else []
                for j in range(4):
                    bp[j]()
                    if ap:
                        ap[j]()
                bp[4]()

    nc.compile()
    return nc


def _host_prep(inputs):
    """Host-side constant preparation (small tensors only)."""
    pos_table = np.asarray(inputs["pos_table"], dtype=np.float32)
    W_pos_w = np.asarray(inputs["W_pos_w"], dtype=np.float32)
    W_pos_b = np.asarray(inputs["W_pos_b"], dtype=np.float32)
    W1_w = np.asarray(inputs["W1_w"], dtype=np.float32)
    W1_b = np.asarray(inputs["W1_b"], dtype=np.float32)
    W2_w = np.asarray(inputs["W2_w"], dtype=np.float32)
    W2_b = np.asarray(inputs["W2_b"], dtype=np.float32)
    q_w = np.asarray(inputs["q_w"], dtype=np.float32)
    q_b = np.asarray(inputs["q_b"], dtype=np.float32)
    rp = np.asarray(inputs["reverse_pos"])

    Wp1 = W_pos_w[:, :H].astype(np.float64)
    Wp2 = W_pos_w[:, H:]
    PC = pos_table.astype(np.float64) @ Wp2.T.astype(np.float64) \
        + W_pos_b.astype(np.float64)                       # [65, H]
    rp_blk = rp[:L].astype(np.int64)
    PC_pos = PC[rp_blk]                                    # [64, H] per position

    # PC fold: hidt' = h + Wp1^-1 @ PC[pos]. Session means are computed on
    # the host from the RAW hidden, so no gate-bias correction is needed.
    corr = np.linalg.solve(Wp1, PC_pos.T)                  # [H, 64] f64

    # col c = i*128 + s*16 + q'  <->  token s*64 + 4*q' + i
    c = np.arange(NT)

    # oh4[p, c] = 1 iff p%32 == sess(c), sess(c) = (c//16)%8: shared moving
    # operand for the 4-way row-tiled z1 broadcast (rows p%32 >= 8 all zero)
    oh4 = np.zeros((H, NT), np.float32)
    p = np.arange(H)
    oh4[(p % 32)[:, None] == ((c // 16) % SPB)[None, :]] = 1.0

    # maskf[q, 32b+s] = 1 iff s < 8 and q//16 == s; tiled 4x over i so the
    # quad_tail STT can read [p, (i b), s] with uniform strides
    maskf = np.zeros((H, H), np.float32)
    q = np.arange(128)
    for b in range(4):
        maskf[q, 32 * b + q // 16] = 1.0
    maskf = np.tile(maskf, (1, 4))

    ret_fp8a = {}
    if FP8_A:
        ret_fp8a["a_dr"] = np.ascontiguousarray(
            Wp1.astype(np.float32).T.reshape(2, 64, H).transpose(1, 0, 2)
        ).astype(FP8)

    cpk = np.concatenate(
        [
            Wp1.astype(np.float32).T,
            W2_w.T.astype(np.float32),
            (W1_w.T / L).astype(np.float32),
            maskf,
            q_w.reshape(H, 1).astype(np.float32),
        ],
        axis=1,
    ).astype(BF16)                                         # [H, 897]
    bbqb = np.concatenate(
        [
            (W1_b + W2_b).reshape(H, 1),
            np.full((H, 1), float(q_b.reshape(-1)[0]), np.float32),
        ],
        axis=1,
    ).astype(np.float32)                                   # [H, 2]
    consts = {
        **ret_fp8a,
        "cpk": cpk,
        "oh4": oh4.astype(BF16),
        "bbqb": bbqb,
    }
    return consts, corr, np.asarray(W1_w, np.float32)


def _prep_hid(hidden, t0, t1, corr, W1f):
    """Natural bf16 + permuted-transposed (PC-folded) copies for [t0, t1),
    plus the host-computed z1 = W1 @ session_mean, packed for the 4-way
    row-tiled broadcast: z1t4[32g+u, k, r, m] = z1[64k+32r+8g+u, m] (u<8)."""
    hc = np.ascontiguousarray(hidden[t0:t1])
    hid_bf = hc.astype(BF16)
    nb = (t1 - t0) // NT
    nsup = nb // SUPER
    mean = hc.reshape(-1, L, H).mean(axis=1, dtype=np.float32)
    z1 = mean @ W1f.T                                      # [nb*SPB, H]
    z1r = z1.reshape(nsup, 2, 4, SPB, H)                   # [k, r, g, u, m]
    z1t4 = np.zeros((4, 32, nsup, 2, H), np.float32)
    z1t4[:, :SPB] = z1r.transpose(2, 3, 0, 1, 4)
    z1t4 = np.ascontiguousarray(
        z1t4.reshape(H, nsup * 2 * H)
    ).astype(BF16)
    X = hc.reshape(nb, 8, 16, 4, H)
    # corr[f, pos], pos = 4*q' + i -> [f, i, q'] addend per (i, q') slot
    corr_iq = corr.reshape(H, 16, 4).transpose(0, 2, 1).astype(np.float32)
    hidt_f = np.ascontiguousarray(
        X.transpose(4, 0, 3, 1, 2).reshape(H, nb, 4, 8, 16)
        + corr_iq[:, None, :, None, :]
    ).reshape(H, nb * NT)
    ret = {"hid": hid_bf, "z1t4": z1t4}
    if FP8_A:
        ret["hidt8"] = hidt_f.reshape(2, 64, nb * NT).transpose(1, 0, 2).astype(FP8)
    else:
        ret["hidt"] = hidt_f.astype(BF16)
    return ret


def _uniform_structure(inputs):
    seq_len = np.asarray(inputs["seq_len"])
    rp = np.asarray(inputs["reverse_pos"])
    if not np.all(seq_len == L):
        return False
    if rp.shape[0] % L != 0:
        return False
    return bool(np.all(rp.reshape(-1, L) == rp[:L]))


def _numpy_fallback(inputs):
    """Exact reference math on host for non-uniform inputs."""
    hidden = np.asarray(inputs["hidden"], np.float32)
    seq_len = np.asarray(inputs["seq_len"])
    rp = np.asarray(inputs["reverse_pos"])
    Bn = seq_len.shape[0]
    seg = np.repeat(np.arange(Bn), seq_len)
    sums = np.zeros((Bn, H), np.float32)
    np.add.at(sums, seg, hidden)
    mean = sums / seq_len[:, None].astype(np.float32)
    pos_emb = np.asarray(inputs["pos_table"], np.float32)[rp]
    W_pos_w = np.asarray(inputs["W_pos_w"], np.float32)
    ph = np.tanh(
        np.concatenate([hidden, pos_emb], -1) @ W_pos_w.T
        + np.asarray(inputs["W_pos_b"], np.float32)
    )
    gate = 1.0 / (
        1.0
        + np.exp(
            -(
                mean[seg] @ np.asarray(inputs["W1_w"], np.float32).T
                + np.asarray(inputs["W1_b"], np.float32)
                + ph @ np.asarray(inputs["W2_w"], np.float32).T
                + np.asarray(inputs["W2_b"], np.float32)
            )
        )
    )
    alpha = gate @ np.asarray(inputs["q_w"], np.float32).T + np.asarray(
        inputs["q_b"], np.float32
    )
    outp = np.zeros((Bn, H), np.float32)
    np.add.at(outp, seg, alpha * hidden)
    return outp


def _ensure_ntff_hook():
    import types

    import antenv

    if "antenv.axon_hooks" not in sys.modules:
        mod = types.ModuleType("antenv.axon_hooks")
        mod._hook = None

        def set_axon_ntff_profile_hook(h, _m=mod):
            _m._hook = h

        def get_axon_ntff_profile_hook(_m=mod):
            return _m._hook

        mod.set_axon_ntff_profile_hook = set_axon_ntff_profile_hook
        mod.get_axon_ntff_profile_hook = get_axon_ntff_profile_hook
        sys.modules["antenv.axon_hooks"] = mod
        antenv.axon_hooks = mod
    import antenv.axon_hooks as ah

    if ah.get_axon_ntff_profile_hook() is None:
        from trn_agent_boot.trn_boot import _ntff_profile_via_ctypes

        hook = _ntff_profile_via_ctypes("/opt/axon/libaxon_pjrt.so")
        if hook is not None:
            ah.set_axon_ntff_profile_hook(hook)


def run(inputs, trace=False, tmpdir=None):
    from concourse import bass_utils

    if trace:
        _ensure_ntff_hook()
        bass_utils.upload_artifacts = lambda d: "local://" + d

    hidden = np.asarray(inputs["hidden"], np.float32)
    T = hidden.shape[0]
    t_core = T // N_CORES
    n_blocks = t_core // NT
    if n_blocks not in _CACHE:
        _CACHE[n_blocks] = _build(n_blocks)
    nc = _CACHE[n_blocks]

    consts, corr, W1f = _host_prep(inputs)
    in_maps = []
    for cix in range(N_CORES):
        m = dict(consts)
        m.update(_prep_hid(hidden, cix * t_core, (cix + 1) * t_core, corr, W1f))
        in_maps.append(m)

    res = bass_utils.run_bass_kernel_spmd(
        nc, in_maps, core_ids=list(range(N_CORES)), trace=trace, tmpdir=tmpdir
    )
    out = np.concatenate([res.results[c]["out"] for c in range(N_CORES)], axis=0)
    return out.astype(np.float32), res


def kernel(**inputs):
    if not _uniform_structure(inputs):
        return _numpy_fallback(inputs)
    out, _ = run(inputs)
    return out



# revision 58
# speedup vs baseline: 1.0007x; 1.0007x over previous
"""Trainium2 Bass kernel for nn_CNNFusing (segment_reduce) — v3.

Math (per token t in session b, H=128, L=64 tokens/session):
  mean_b   = (1/L) sum_{t in b} hidden_t
  pos_h_t  = tanh(hidden_t @ Wp1.T + pos_table[rp_t] @ Wp2.T + W_pos_b)
  gate_t   = sigmoid(mean_b @ W1.T + W1_b + pos_h_t @ W2.T + W2_b)
  alpha_t  = gate_t @ q_w.T + q_b
  out_b    = sum_{t in b} alpha_t * hidden_t

Design (measured ~173 us on 8 axon trn2 cores, rel err ~7.7e-3; the
v2 baseline was ~225 us):
  - hidT is transposed on the HOST (no hardware DMA-transpose) with a
    per-block column permutation col = i*128 + s*16 + q' <-> token
    t = s*64 + 4*q' + i, so that:
      * natural-layout loads are fully contiguous (partition p holds 4
        consecutive tokens 4p..4p+3 => 1 KiB runs, full DMA rate)
      * alpha chunk i is a contiguous 128-col stationary slice of gate
        whose output partition order matches the natural-layout partitions
  - The position-table add is folded into the A matmul on the host:
    hidt' = h + Wp1^-1 @ PC[pos].
  - Session means, and hence z1 = W1 @ mean, are computed on the host
    (fp32, improves accuracy) and DMA'd in packed for a 4-WAY ROW-TILED
    z1 broadcast: each block's z1 only needs its 8 sessions as
    contraction, so the 4 blocks of a quad use the four 32-row PE
    groups and their onehot matmuls run concurrently.
  - hid and hidT stream on SEPARATE DMA queues (sync / gpsimd): one
    queue saturates at ~260 GB/s, the pair needs ~360.
  - alpha matmuls write (i*4+b)-ordered columns so ONE fused DVE
    scalar_tensor_tensor per quad builds the masked wsum stationary
    (3-dim APs only — the STT verifier rejects 4-dim).
  - outputs are staged in SBUF, stored once per superblock on the sync
    queue (NOT behind the hidT loads on gpsimd).
  - The engine schedule is ACT-bound (tanh+sigmoid = 2 passes over
    [T, H] at 1 elem/lane/cycle ~= 110 us + per-call overhead): the
    main loop interleaves B(k) parts BEFORE A(k+1) parts so sigmoids
    (which release PSUM tiles and feed the alphas) never queue behind
    a burst of tanhs on the FIFO ACT engine.
  - PE pre-warm via a memset-backed stationary (no DMA dependency)
    covers the HAM clock-gate window during the DMA-queue ramp.
"""

import sys

sys.path.insert(0, "/opt/trn_rl_repo")

import numpy as np
import ml_dtypes

BF16 = ml_dtypes.bfloat16
FP8 = ml_dtypes.float8_e4m3

H = 128
L = 64
NT = 512            # tokens per block
SPB = 8             # sessions per block
SUPER = 8           # blocks per superblock
N_CORES = 8
FP8_A = False  # fp8 DoubleRow for the Wp1 (A) matmul: measured 5.5e-2 rel err
               # (fails 2e-2 gate) and slower than bf16+FWL — keep off

_CACHE = {}


def _build(n_blocks):
    import concourse.bacc as bacc
    import concourse.bass as bass
    import concourse.tile as tile
    from concourse import mybir

    f32 = mybir.dt.float32
    bf16 = mybir.dt.bfloat16
    fp8 = mybir.dt.float8e4
    DR = mybir.MatmulPerfMode.DoubleRow
    Tanh = mybir.ActivationFunctionType.Tanh
    Sig = mybir.ActivationFunctionType.Sigmoid
    X = mybir.AxisListType.X
    XY = mybir.AxisListType.XY
    ADD = mybir.AluOpType.add
    MUL = mybir.AluOpType.mult

    T_core = n_blocks * NT
    assert n_blocks % SUPER == 0
    n_super = n_blocks // SUPER

    nc = bacc.Bacc("TRN2", target_bir_lowering=False, debug=False)

    hid = nc.dram_tensor("hid", [T_core, H], bf16, kind="ExternalInput").ap()
    if FP8_A:
        hidt8 = nc.dram_tensor("hidt8", [64, 2, T_core], fp8, kind="ExternalInput").ap()
        a_drt = nc.dram_tensor("a_dr", [64, 2, H], fp8, kind="ExternalInput").ap()
    else:
        hidt = nc.dram_tensor("hidt", [H, T_core], bf16, kind="ExternalInput").ap()
    z1t4 = nc.dram_tensor(
        "z1t4", [H, n_super * 2 * H], bf16, kind="ExternalInput"
    ).ap()
    cpk = nc.dram_tensor("cpk", [H, 897], bf16, kind="ExternalInput").ap()
    oh4 = nc.dram_tensor("oh4", [H, NT], bf16, kind="ExternalInput").ap()
    bbqb = nc.dram_tensor("bbqb", [H, 2], f32, kind="ExternalInput").ap()
    out = nc.dram_tensor("out", [n_blocks * SPB, H], f32, kind="ExternalOutput").ap()

    with tile.TileContext(nc) as tc:
        with (
            tc.tile_pool(name="consts", bufs=1) as consts,
            tc.tile_pool(name="hidn", bufs=7) as hidn_pool,
            tc.tile_pool(name="hidtp", bufs=5) as hidt_pool,
            tc.tile_pool(name="posh", bufs=12) as posh_pool,
            tc.tile_pool(name="gate", bufs=6) as gate_pool,
            tc.tile_pool(name="smallsb", bufs=3) as smallsb,
            tc.tile_pool(name="acbp", bufs=2) as acb_pool,
            tc.tile_pool(name="osbp", bufs=2) as osb_pool,
            tc.tile_pool(name="dense_ps", bufs=3, space=bass.MemorySpace.PSUM) as dense_ps,
            tc.tile_pool(name="out_ps", bufs=1, space=bass.MemorySpace.PSUM) as out_ps,
            tc.tile_pool(name="small_ps", bufs=1, space=bass.MemorySpace.PSUM) as small_ps,
        ):
    # ---- constants ----
            # PE pre-warm first: a memset-backed stationary means the warm
            # burst has NO DMA dependency and starts in the preamble shadow.
            warm_sb = consts.tile([1, 1], f32)
            wz_sb = consts.tile([H, H], bf16)
            nc.vector.memset(wz_sb, 0.0)
            warm_ps = dense_ps.tile([128, 2, NT], f32, tag="dense")
            for i in range(60):
                nc.tensor.matmul(
                    warm_ps[:, i % 2, 0:128], wz_sb, wz_sb,
                    start=True, stop=True,
                )
            nc.vector.tensor_copy(warm_sb, warm_ps[0:1, 0, 0:1])

            # one packed bf16 const DMA (a | w2 | w1 | mf16 | q), plus three
            # small ones: few issue slots, single descriptor streams
            cpk_sb = consts.tile([H, 897], bf16)
            nc.scalar.dma_start(cpk_sb, cpk)
            a_sb = cpk_sb[:, 0:128]
            w2_sb = cpk_sb[:, 128:256]
            w1_sb = cpk_sb[:, 256:384]
            mf16_sb = cpk_sb[:, 384:896]
            q_sb = cpk_sb[:, 896:897]
            oh4_sb = consts.tile([H, NT], bf16)
            nc.scalar.dma_start(oh4_sb, oh4)
            z1t4_sb = consts.tile([H, n_super, 2, H], bf16)
            nc.scalar.dma_start(
                z1t4_sb.rearrange("p k r m -> p (k r m)"), z1t4
            )
            bbqb_sb = consts.tile([H, 2], f32)
            nc.scalar.dma_start(bbqb_sb, bbqb)
            bb_sb = bbqb_sb[:, 0:1]
            qb_sb = bbqb_sb[:, 1:2]
            if FP8_A:
                adr_sb = consts.tile([64, 2, H], fp8)
                nc.scalar.dma_start(
                    adr_sb.rearrange("p k h -> p (k h)"),
                    a_drt.rearrange("p k h -> p (k h)"),
                )

            state = {}

            def phase_a_parts(k):
                st = {"hidnats": [], "poshs": [], "ht": None}
                state[k] = st

                def mk(qd, pr):
                    def part():
                        if pr == 0:
                            t0 = (k * SUPER + qd * 4) * NT
                            hidnat = hidn_pool.tile([128, 4, 4, H], bf16, tag="hn")
                            nc.sync.dma_start(
                                hidnat.rearrange("p b i h -> p b (i h)"),
                                hid[t0:t0 + 4 * NT, :].rearrange(
                                    "(b p i) h -> p b (i h)", p=128, i=4
                                ),
                            )
                            st["hidnats"].append(hidnat)
                            # separate hardware DMA queue from hidnat's (sync):
                            # one queue saturates at ~260 GB/s
                            if FP8_A:
                                ht = hidt_pool.tile([64, 2, 4 * NT], fp8, tag="ht")
                                nc.gpsimd.dma_start(
                                    ht, hidt8[:, :, t0:t0 + 4 * NT]
                                )
                            else:
                                ht = hidt_pool.tile([H, 4 * NT], bf16, tag="ht")
                                if k == 0 and qd == 0:
                                    # split the very first load so the A
                                    # matmul of block 0 starts ~3us earlier
                                    for blk in range(4):
                                        nc.gpsimd.dma_start(
                                            ht[:, blk * NT:(blk + 1) * NT],
                                            hidt[:, t0 + blk * NT:t0 + (blk + 1) * NT],
                                        )
                                else:
                                    nc.gpsimd.dma_start(
                                        ht, hidt[:, t0:t0 + 4 * NT]
                                    )
                            st["ht"] = ht
                        hidt_sb = st["ht"]
                        zpair = dense_ps.tile([128, 2, NT], f32, tag="dense")
                        for b2 in range(2):
                            off = (pr * 2 + b2) * NT
                            if FP8_A:
                                nc.tensor.matmul(
                                    zpair[:, b2, :], adr_sb,
                                    hidt_sb[:, :, off:off + NT],
                                    start=True, stop=True, perf_mode=DR,
                                )
                            else:
                                nc.tensor.matmul(
                                    zpair[:, b2, :], a_sb,
                                    hidt_sb[:, off:off + NT],
                                    start=True, stop=True,
                                )
                        posh = posh_pool.tile([128, 2, NT], bf16)
                        nc.scalar.activation(posh, zpair, Tanh)
                        st["poshs"].append(posh)
                    return part

                return [mk(qd, pr) for qd in range(2) for pr in range(2)]

            def phase_b_parts(k):
                st = state[k]
                st["pend"] = []
                st["gates"] = {}
                st["zp"] = {}

                def alphas(qd, pr):
                    gate, ab = st["gates"][(qd, pr)]
                    for b2 in range(2):
                        b = pr * 2 + b2  # block in quad
                        for i in range(4):
                            # ab column order (i, b) so the quad_tail STT can
                            # view ab as one uniform-stride (i b) dim
                            nc.tensor.matmul(
                                ab[:, i * 4 + b:i * 4 + b + 1],
                                gate[:, b2, i * 128:(i + 1) * 128], q_sb,
                                start=True, stop=True,
                            )

                def quad_tail(qd):
                    ab = st["abps"][qd]
                    acb = acb_pool.tile([128, 4, H], bf16, tag="acb")
                    # one fused STT for the whole quad: (alpha + qb) * mask.
                    # ab cols are (i b)-ordered and acb's (i, b, s) layout is
                    # contiguous, so both sides collapse to 3-dim APs (the
                    # STT verifier rejects 4-dim ones); mf16_sb pre-tiles the
                    # mask over i.
                    nc.vector.scalar_tensor_tensor(
                        acb.rearrange("p i (b s) -> p (i b) s", b=4),
                        ab[:, :, None].broadcast_to((128, 16, 32)),
                        qb_sb[:, 0:1],
                        mf16_sb.rearrange("p (ib s) -> p ib s", s=32),
                        op0=ADD, op1=MUL,
                    )
                    out4 = out_ps.tile([128, 4, H], f32)
                    for i in range(4):
                        nc.tensor.matmul(
                            out4, acb[:, i, :], st["hidnats"][qd][:, :, i, :],
                            start=(i == 0), stop=(i == 3),
                        )
                    for b in range(4):
                        nc.vector.tensor_copy(
                            st["osb"][32 * b:32 * b + SPB, qd, :],
                            out4[32 * b:32 * b + SPB, b, :],
                        )

                def flush_one():
                    pq = st["pend"].pop(0)
                    alphas(*pq)
                    if pq[1] == 1:
                        quad_tail(pq[0])

                def mk(qd, pr):
                    def part():
                        if qd == 0 and pr == 0:
                            st["osb"] = osb_pool.tile([128, 2, H], f32, name="osb")
                            ab2 = small_ps.tile([128, 32], f32, tag="ab")
                            st["abps"] = [ab2[:, 0:16], ab2[:, 16:32]]
                        posh = st["poshs"][qd * 2 + pr]
                        zpair2 = dense_ps.tile([128, 2, NT], f32, tag="dense")
                        for b2 in range(2):
                            nc.tensor.matmul(
                                zpair2[:, b2, :], w2_sb, posh[:, b2, :],
                                start=True, stop=False,
                            )
                        st["zp"][pr] = zpair2
                        if pr == 1:
                            # 4-way row-tiled z1 broadcast: each block's z1
                            # needs only its 8 sessions as contraction, so
                            # the four blocks of the quad go to the four
                            # 32-row PE groups and run concurrently
                            for g in range(4):
                                prg, b2g = g // 2, g % 2
                                nc.tensor.matmul(
                                    st["zp"][prg][:, b2g, :],
                                    z1t4_sb[32 * g:32 * g + 32, k, qd, :],
                                    oh4_sb[32 * g:32 * g + 32, :],
                                    start=False, stop=True,
                                    tile_position=(32 * g, 0),
                                )
                            for prg in range(2):
                                gate = gate_pool.tile([128, 2, NT], bf16)
                                nc.scalar.activation(
                                    gate, st["zp"][prg], Sig, bias=bb_sb
                                )
                                st["gates"][(qd, prg)] = (gate, st["abps"][qd])
                                st["pend"].append((qd, prg))
                            # eager flush on the last super (no A-phase work
                            # left to hide the alpha/wsum latency behind)
                            lag = 0 if k == n_super - 1 else 1
                            while len(st["pend"]) > lag:
                                flush_one()
                    return part

                def tail():
                    while st["pend"]:
                        flush_one()
                    # sync queue: the gpsimd queue still has hidt loads of
                    # later superblocks in flight; don't queue stores behind
                    # them (the final drain was eating ~10us)
                    ov = out[k * 64:(k + 1) * 64, :].rearrange(
                        "(h g2 s) x -> g2 s h x", h=2, g2=4
                    )
                    for g in range(4):
                        nc.sync.dma_start(
                            ov[g], st["osb"][32 * g:32 * g + SPB, :, :]
                        )
                    del state[k]

                return [mk(qd, pr) for qd in range(2) for pr in range(2)] + [tail]

            # software pipeline with pair-level interleave: A(k+1) pairs
            # alternate with B(k) pairs so every engine sees a steady mix
            # interleave B(k) parts with A(k+1) parts, B first: sigmoids
            # (critical path: they release PSUM tiles and feed the alphas)
            # must not queue behind a burst of tanhs on the FIFO ACT engine
            for p in phase_a_parts(0):
                p()
            for k in range(n_super):
                bp = phase_b_parts(k)
                ap = phase_a_parts(k + 1) if k + 1 < n_super else []
                for j in range(4):
                    bp[j]()
                    if ap:
                        ap[j]()
                bp[4]()

    nc.compile()
    return nc


def _host_prep(inputs):
    """Host-side constant preparation (small tensors only)."""
    pos_table = np.asarray(inputs["pos_table"], dtype=np.float32)
    W_pos_w = np.asarray(inputs["W_pos_w"], dtype=np.float32)
    W_pos_b = np.asarray(inputs["W_pos_b"], dtype=np.float32)
    W1_w = np.asarray(inputs["W1_w"], dtype=np.float32)
    W1_b = np.asarray(inputs["W1_b"], dtype=np.float32)
    W2_w = np.asarray(inputs["W2_w"], dtype=np.float32)
    W2_b = np.asarray(inputs["W2_b"], dtype=np.float32)
    q_w = np.asarray(inputs["q_w"], dtype=np.float32)
    q_b = np.asarray(inputs["q_b"], dtype=np.float32)
    rp = np.asarray(inputs["reverse_pos"])

    Wp1 = W_pos_w[:, :H].astype(np.float64)
    Wp2 = W_pos_w[:, H:]
    PC = pos_table.astype(np.float64) @ Wp2.T.astype(np.float64) \
        + W_pos_b.astype(np.float64)                       # [65, H]
    rp_blk = rp[:L].astype(np.int64)
    PC_pos = PC[rp_blk]                                    # [64, H] per position

    # PC fold: hidt' = h + Wp1^-1 @ PC[pos]. Session means are computed on
    # the host from the RAW hidden, so no gate-bias correction is needed.
    corr = np.linalg.solve(Wp1, PC_pos.T)                  # [H, 64] f64

    # col c = i*128 + s*16 + q'  <->  token s*64 + 4*q' + i
    c = np.arange(NT)

    # oh4[p, c] = 1 iff p%32 == sess(c), sess(c) = (c//16)%8: shared moving
    # operand for the 4-way row-tiled z1 broadcast (rows p%32 >= 8 all zero)
    oh4 = np.zeros((H, NT), np.float32)
    p = np.arange(H)
    oh4[(p % 32)[:, None] == ((c // 16) % SPB)[None, :]] = 1.0

    # maskf[q, 32b+s] = 1 iff s < 8 and q//16 == s; tiled 4x over i so the
    # quad_tail STT can read [p, (i b), s] with uniform strides
    maskf = np.zeros((H, H), np.float32)
    q = np.arange(128)
    for b in range(4):
        maskf[q, 32 * b + q // 16] = 1.0
    maskf = np.tile(maskf, (1, 4))

    ret_fp8a = {}
    if FP8_A:
        ret_fp8a["a_dr"] = np.ascontiguousarray(
            Wp1.astype(np.float32).T.reshape(2, 64, H).transpose(1, 0, 2)
        ).astype(FP8)

    cpk = np.concatenate(
        [
            Wp1.astype(np.float32).T,
            W2_w.T.astype(np.float32),
            (W1_w.T / L).astype(np.float32),
            maskf,
            q_w.reshape(H, 1).astype(np.float32),
        ],
        axis=1,
    ).astype(BF16)                                         # [H, 897]
    bbqb = np.concatenate(
        [
            (W1_b + W2_b).reshape(H, 1),
            np.full((H, 1), float(q_b.reshape(-1)[0]), np.float32),
        ],
        axis=1,
    ).astype(np.float32)                                   # [H, 2]
    consts = {
        **ret_fp8a,
        "cpk": cpk,
        "oh4": oh4.astype(BF16),
        "bbqb": bbqb,
    }
    return consts, corr, np.asarray(W1_w, np.float32)


def _prep_hid(hidden, t0, t1, corr, W1f):
    """Natural bf16 + permuted-transposed (PC-folded) copies for [t0, t1),
    plus the host-computed z1 = W1 @ session_mean, packed for the 4-way
    row-tiled broadcast: z1t4[32g+u, k, r, m] = z1[64k+32r+8g+u, m] (u<8)."""
    hc = np.ascontiguousarray(hidden[t0:t1])
    hid_bf = hc.astype(BF16)
    nb = (t1 - t0) // NT
    nsup = nb // SUPER
    mean = hc.reshape(-1, L, H).mean(axis=1, dtype=np.float32)
    z1 = mean @ W1f.T                                      # [nb*SPB, H]
    z1r = z1.reshape(nsup, 2, 4, SPB, H)                   # [k, r, g, u, m]
    z1t4 = np.zeros((4, 32, nsup, 2, H), np.float32)
    z1t4[:, :SPB] = z1r.transpose(2, 3, 0, 1, 4)
    z1t4 = np.ascontiguousarray(
        z1t4.reshape(H, nsup * 2 * H)
    ).astype(BF16)
    X = hc.reshape(nb, 8, 16, 4, H)
    # corr[f, pos], pos = 4*q' + i -> [f, i, q'] addend per (i, q') slot
    corr_iq = corr.reshape(H, 16, 4).transpose(0, 2, 1).astype(np.float32)
    hidt_f = np.ascontiguousarray(
        X.transpose(4, 0, 3, 1, 2).reshape(H, nb, 4, 8, 16)
        + corr_iq[:, None, :, None, :]
    ).reshape(H, nb * NT)
    ret = {"hid": hid_bf, "z1t4": z1t4}
    if FP8_A:
        ret["hidt8"] = hidt_f.reshape(2, 64, nb * NT).transpose(1, 0, 2).astype(FP8)
    else:
        ret["hidt"] = hidt_f.astype(BF16)
    return ret


def _uniform_structure(inputs):
    seq_len = np.asarray(inputs["seq_len"])
    rp = np.asarray(inputs["reverse_pos"])
    if not np.all(seq_len == L):
        return False
    if rp.shape[0] % L != 0:
        return False
    return bool(np.all(rp.reshape(-1, L) == rp[:L]))


def _numpy_fallback(inputs):
    """Exact reference math on host for non-uniform inputs."""
    hidden = np.asarray(inputs["hidden"], np.float32)
    seq_len = np.asarray(inputs["seq_len"])
    rp = np.asarray(inputs["reverse_pos"])
    Bn = seq_len.shape[0]
    seg = np.repeat(np.arange(Bn), seq_len)
    sums = np.zeros((Bn, H), np.float32)
    np.add.at(sums, seg, hidden)
    mean = sums / seq_len[:, None].astype(np.float32)
    pos_emb = np.asarray(inputs["pos_table"], np.float32)[rp]
    W_pos_w = np.asarray(inputs["W_pos_w"], np.float32)
    ph = np.tanh(
        np.concatenate([hidden, pos_emb], -1) @ W_pos_w.T
        + np.asarray(inputs["W_pos_b"], np.float32)
    )
    gate = 1.0 / (
        1.0
        + np.exp(
            -(
                mean[seg] @ np.asarray(inputs["W1_w"], np.float32).T
                + np.asarray(inputs["W1_b"], np.float32)
                + ph @ np.asarray(inputs["W2_w"], np.float32).T
                + np.asarray(inputs["W2_b"], np.float32)
            )
        )
    )
    alpha = gate @ np.asarray(inputs["q_w"], np.float32).T + np.asarray(
        inputs["q_b"], np.float32
    )
    outp = np.zeros((Bn, H), np.float32)
    np.add.at(outp, seg, alpha * hidden)
    return outp


def _ensure_ntff_hook():
    import types

    import antenv

    if "antenv.axon_hooks" not in sys.modules:
        mod = types.ModuleType("antenv.axon_hooks")
        mod._hook = None

        def set_axon_ntff_profile_hook(h, _m=mod):
            _m._hook = h

        def get_axon_ntff_profile_hook(_m=mod):
            return _m._hook

        mod.set_axon_ntff_profile_hook = set_axon_ntff_profile_hook
        mod.get_axon_ntff_profile_hook = get_axon_ntff_profile_hook
        sys.modules["antenv.axon_hooks"] = mod
        antenv.axon_hooks = mod
    import antenv.axon_hooks as ah

    if ah.get_axon_ntff_profile_hook() is None:
        from trn_agent_boot.trn_boot import _ntff_profile_via_ctypes

        hook = _ntff_profile_via_ctypes("/opt/axon/libaxon_pjrt.so")
        if hook is not None:
            ah.set_axon_ntff_profile_hook(hook)


def run(inputs, trace=False, tmpdir=None):
    from concourse import bass_utils

    if trace:
        _ensure_ntff_hook()
        bass_utils.upload_artifacts = lambda d: "local://" + d

    hidden = np.asarray(inputs["hidden"], np.float32)
    T = hidden.shape[0]
    t_core = T // N_CORES
    n_blocks = t_core // NT
    if n_blocks not in _CACHE:
        _CACHE[n_blocks] = _build(n_blocks)
    nc = _CACHE[n_blocks]

    consts, corr, W1f = _host_prep(inputs)
    in_maps = []
    for cix in range(N_CORES):
        m = dict(consts)
        m.update(_prep_hid(hidden, cix * t_core, (cix + 1) * t_core, corr, W1f))
        in_maps.append(m)

    res = bass_utils.run_bass_kernel_spmd(
        nc, in_maps, core_ids=list(range(N_CORES)), trace=trace, tmpdir=tmpdir
    )
    out = np.concatenate([res.results[c]["out"] for c in range(N_CORES)], axis=0)
    return out.astype(np.float32), res


def kernel(**inputs):
    if not _uniform_structure(inputs):
        return _numpy_fallback(inputs)
    out, _ = run(inputs)
    return out

